# revision 2
# baseline (speedup 1.0000x reference)
"""Trainium2 Bass kernel for nn_LuongAttention.

Reference math (per batch b):
    S   = Dec @ Enc^T          # [T_dec, T_enc]
    Out = S @ Enc              # [T_dec, D]

By associativity:  Out = Dec @ (Enc^T @ Enc) = Dec @ G with G = Enc^T Enc
a [D, D] = [128, 128] Gram matrix.  This removes the [2048, 2048]
intermediate entirely (16x less FLOPs) and makes the kernel
memory-bound: ~1.5 MiB HBM I/O per core at fp16.

Sharding: data-parallel over batch B=8 -> one batch per NeuronCore.

Device-side layout trick: the host feeds Dec pre-transposed (DecT
[D, T]) and receives Out transposed (OutT [D, T]); the host transposes
the result back during the gather (pure layout permutation, no math).
With that:
  - G = sum_i EncTile_i^T @ EncTile_i  (accumulating PE matmuls, natural
    encoder layout - no transposes needed)
  - OutT = G @ DecT computed as matmul(lhsT=G, rhs=DecT chunk) with wide
    moving chunks (G is symmetric so lhsT=G gives G.T@X = G@X)
  - no PE transposes, no identity, minimal PSUM->SBUF copies

v2 schedule (from the v1 trace):
  - ALL DMA on the two HWDGE rings (sync/SP + scalar/ACT).  v1 put DecT
    on the SWDGE (gpsimd) queue, whose Q7 ring-drain kept every engine
    in the end-of-kernel barrier ~9 us after the last real work.
  - enc chunks are issued FIRST on both rings, dect behind them:
    HWDGE rings are FIFO and SDMA round-robins between rings packet-
    wise, so this lands the G-gating encoder at full HBM bandwidth
    instead of finishing together with everything else.
  - enc split in ENC_CHUNKS so Gram matmuls start when the first chunk
    lands, not when the full tensor does.
  - junk warm-up matmuls run during the load phase so the PE HAM clock
    gate (1.2 -> 2.4 GHz after ~3.4 us of activity) releases in time
    for the final matmuls.
  - final phase per 512-col chunk: PE matmul -> PSUM->SBUF cast copy
    (alternating DVE/ACT) -> store (alternating rings, queued after
    each ring's loads).
"""

import os
import sys
from contextlib import ExitStack

import numpy as np

for _p in (
    "/opt/trn_rl_repo",
    "/root/.axon_site",
    "/root/.axon_site/_ro/trn_rl_repo",
    "/root/.axon_site/_ro/pypackages",
):
    if os.path.isdir(_p) and _p not in sys.path:
        sys.path.append(_p)

import concourse.bacc as bacc
import concourse.mybir as mybir
import concourse.tile as tile
from concourse.bass_utils import run_bass_kernel_spmd

B, T, D, P = 8, 2048, 128, 128
NT = T // P  # 16 row tiles of 128

# tunables
MM_DTYPE = "fp16"  # "fp32" | "bf16" | "fp16"
ENC_CHUNKS = 4  # enc tiles split across both rings, issued first
DEC_CHUNKS = 2  # dect loads, issued behind enc on both rings
FINAL_N = 512  # moving-operand width of the final matmul (1 PSUM bank)
OUT_FP16 = True  # store OutT as fp16; host upcasts to fp32 after gather
WARMUP_MMS = 4  # junk matmuls issued early to trigger the PE HAM clock ramp


def _build_nc(mm_dtype=None):
    mm_dtype = mm_dtype or MM_DTYPE
    nc = bacc.Bacc("TRN2", target_bir_lowering=False, debug=False)
    f32 = mybir.dt.float32
    bf16 = mybir.dt.bfloat16
    fp16 = mybir.dt.float16

    in_dt = {"bf16": bf16, "fp16": fp16}.get(mm_dtype, f32)

    # enc arrives host-pre-shuffled to the SBUF layout [p, n*d] so chunk
    # loads are contiguous per partition.
    enc_h = nc.dram_tensor("enc", [P, NT * D], in_dt, kind="ExternalInput")
    dect_h = nc.dram_tensor("dect", [D, T], in_dt, kind="ExternalInput")
    out_dt = fp16 if OUT_FP16 else f32
    out_h = nc.dram_tensor("out", [D, T], out_dt, kind="ExternalOutput")

    # [p, n, d] view of encoder (p = row within tile, n = tile index)
    enc_v = enc_h.ap().rearrange("p (n d) -> p n d", d=D)
    dect_v = dect_h.ap()
    out_v = out_h.ap()

    rings = [nc.sync, nc.scalar]

    with ExitStack() as ctx:
        tc = ctx.enter_context(tile.TileContext(nc))
        singles = ctx.enter_context(tc.tile_pool(name="singles", bufs=1))
        psum = ctx.enter_context(tc.tile_pool(name="psum", bufs=4, space="PSUM"))
        gpsum = ctx.enter_context(tc.tile_pool(name="gpsum", bufs=1, space="PSUM"))

        enc_sb = singles.tile([P, NT, D], in_dt)
        dect_sb = singles.tile([P, T], in_dt)
        out_sb = singles.tile([P, T], out_dt)

        # ---- loads: enc first on both rings, dect behind ----
        base, rem = divmod(NT, ENC_CHUNKS)
        sizes = [base + (1 if c < rem else 0) for c in range(ENC_CHUNKS)]
        pos = 0
        enc_bounds = []  # tile-index upper bound per chunk
        for c, sz in enumerate(sizes):
            rings[c % 2].dma_start(
                out=enc_sb[:, pos : pos + sz, :],
                in_=enc_v[:, pos : pos + sz, :],
            )
            pos += sz
            enc_bounds.append(pos)
        cs = T // DEC_CHUNKS
        for c in range(DEC_CHUNKS):
            rings[c % 2].dma_start(
                out=dect_sb[:, c * cs : (c + 1) * cs],
                in_=dect_v[:, c * cs : (c + 1) * cs],
            )

        # ---- PE warm-up during the load phase ----
        if WARMUP_MMS:
            wsrc = singles.tile([P, 512], in_dt)
            nc.vector.memset(wsrc[:], 0.0)
            wps = gpsum.tile([P, 512], f32, tag="warm")
            for w in range(WARMUP_MMS):
                nc.tensor.matmul(
                    wps[:],
                    lhsT=wsrc[:, :P],
                    rhs=wsrc[:],
                    start=(w == 0),
                    stop=(w == WARMUP_MMS - 1),
                )

        # ---- Gram matrix construction (chunk-pipelined behind enc DMAs) ----
        g_sb = singles.tile([P, P], in_dt)
        g_ps = gpsum.tile([P, P], f32, tag="ga")
        for i in range(NT):
            nc.tensor.matmul(
                g_ps[:],
                lhsT=enc_sb[:, i, :],
                rhs=enc_sb[:, i, :],
                start=(i == 0),
                stop=(i == NT - 1),
            )
        nc.vector.tensor_copy(g_sb[:], g_ps[:])

        # ---- OutT = G @ DecT: wide moving chunks, stationary G ----
        # Pipeline per chunk: PE matmul -> (DVE|ACT) PSUM->SBUF cast ->
        # store on (sync|scalar), queued behind that ring's loads.
        n_final = T // FINAL_N
        for c in range(n_final):
            op = psum.tile([P, FINAL_N], f32, tag="op")
            lo = c * FINAL_N
            nc.tensor.matmul(
                op[:],
                lhsT=g_sb[:],
                rhs=dect_sb[:, lo : lo + FINAL_N],
                start=True,
                stop=True,
            )
            if c % 2 == 0:
                nc.vector.tensor_copy(out_sb[:, lo : lo + FINAL_N], op[:])
            else:
                nc.scalar.copy(out_sb[:, lo : lo + FINAL_N], op[:])
            rings[c % 2].dma_start(
                out=out_v[:, lo : lo + FINAL_N],
                in_=out_sb[:, lo : lo + FINAL_N],
            )

    nc.compile()
    return nc


_NC = {}


def _get_nc(mm_dtype=None):
    mm_dtype = mm_dtype or MM_DTYPE
    if mm_dtype not in _NC:
        _NC[mm_dtype] = _build_nc(mm_dtype)
    return _NC[mm_dtype]


def _np_in_dtype(mm_dtype):
    if mm_dtype == "bf16":
        import ml_dtypes

        return ml_dtypes.bfloat16
    if mm_dtype == "fp16":
        return np.float16
    return np.float32


def _run(enc, dec, mm_dtype=None, **kwargs):
    mm_dtype = mm_dtype or MM_DTYPE
    nc = _get_nc(mm_dtype)
    np_dt = _np_in_dtype(mm_dtype)
    in_maps = []
    for b in range(B):
        in_maps.append(
            {
                "enc": np.ascontiguousarray(
                    enc[b].astype(np_dt).reshape(NT, P, D).transpose(1, 0, 2).reshape(P, NT * D)
                ),
                "dect": np.ascontiguousarray(dec[b].T.astype(np_dt)),
            }
        )
    res = run_bass_kernel_spmd(nc, in_maps, core_ids=list(range(B)), **kwargs)
    out = np.stack([res.results[b]["out"].T.astype(np.float32) for b in range(B)], axis=0)
    return np.ascontiguousarray(out), res


def kernel(encoder_hidden_states, decoder_hidden_states):
    enc = np.ascontiguousarray(np.asarray(encoder_hidden_states, dtype=np.float32))
    dec = np.ascontiguousarray(np.asarray(decoder_hidden_states, dtype=np.float32))
    assert enc.shape == (B, T, D) and dec.shape == (B, T, D)
    out, _ = _run(enc, dec)
    return out
